# revision 37
# baseline (speedup 1.0000x reference)
"""COMA loss kernel for Trainium2 — v5: col-tiled concurrent PE reductions.

Layout per core (B sharded 8 ways, BL=16, BA=BL*A=128 rows):
  ba = 64*h + j  (h in {0,1}, j in [0,64))
  SBUF partition p = 64*h + n   (n = action index, N=64)
  free index     f = j*T + t    (F = 64*T = 16384), j-major

Six per-(ba,t) reductions over n (partitions): all six product tensors
stream through the PE concurrently via 32-column tile_position packing —
ONE shared [128,2] h-mask stationary, each product's matmul writes a
2-row slice at col group {0,32,64,96} of a PSUM tile (wave1: 4 products,
wave2: 2).  No stationary reloads between matmuls, waves pipeline.

PSUM is evacuated by scalar-engine copies (cost = free size only), the
data rows are bounced to DRAM with partition-strided DMAs and read back
transposed into the stage-2 layout s2d[j, h, rho, t].

The onehot is a 4x-mode tensor_scalar is_equal against a per-partition
iota; the replicated action tensor is produced on-chip by broadcast
DMAs from a compact [2, F] HBM tensor.  DVE carries four of the five
fp16 products (the fifth runs on the idle GPSIMD) plus stage 2.
"""

import sys

for _p in ("/opt/trn_rl_repo",):
    if _p not in sys.path:
        sys.path.insert(0, _p)

import numpy as np

import concourse.bass as bass
import concourse.bacc as bacc
import concourse.mybir as mybir
from concourse.bass_utils import run_bass_kernel_spmd
from concourse.tile import TileContext

T, B, A, N = 256, 128, 8, 64
M = 8                 # cores
BL = B // M
BA = BL * A           # 128
H, J = 2, 64          # ba = 64h + j
F = J * T             # 16384: f = j*T + t
CB = 4096             # big load chunk (j-block of 16)
NCB = F // CB         # 4
FCH = 2048            # compute chunk (j-block of 8)
NCH = F // FCH        # 8
SUB = 512             # matmul f-subchunk (one PSUM bank column span)
GAMMA, LAMBDA = 0.99, 0.95

F32 = mybir.dt.float32
F16 = mybir.dt.float16

# rho order fixed by PSUM col-group packing: wave1 = cols {0,32,64,96},
# wave2 = cols {0,32} of the second tile.
R_SUME, R_DOTEQ, R_QTK, R_TQTK, R_DOTEL, R_LTK = range(6)


def build_program() -> bass.Bass:
    nc = bacc.Bacc("TRN2", target_bir_lowering=False, debug=False)

    lg_d = nc.dram_tensor("logit", [BA, F], F16, kind="ExternalInput")
    qv_d = nc.dram_tensor("qv", [BA, F], F16, kind="ExternalInput")
    tqv_d = nc.dram_tensor("tqv", [BA, F], F16, kind="ExternalInput")
    ohf_d = nc.dram_tensor("ohf", [BA, F], F16, kind="ExternalInput")
    hmask_d = nc.dram_tensor("hmask", [BA, 2], F16, kind="ExternalInput")
    wgt_d = nc.dram_tensor("wgt", [J, H * T], F16, kind="ExternalInput")
    rwd_d = nc.dram_tensor("rwd", [J, H * T], F16, kind="ExternalInput")
    out_d = nc.dram_tensor("out", [J, 3], F32, kind="ExternalOutput")

    OP = mybir.AluOpType
    AF = mybir.ActivationFunctionType

    with TileContext(nc) as tc:
        with (
            tc.tile_pool(name="inp", bufs=2) as inp,
            tc.tile_pool(name="scr", bufs=3) as scr,
            tc.tile_pool(name="evc", bufs=2) as evc,
            tc.tile_pool(name="per", bufs=1) as per,
            tc.tile_pool(name="ps1", bufs=2, space=bass.MemorySpace.PSUM) as ps1,
            tc.tile_pool(name="drb", bufs=4, space="DRAM") as drb,
        ):
            # ---- constants / small inputs ---------------------------------
            # (small setup loads issued from scalar so the first big input
            # DMAs own the sync queue immediately)
            hmask = per.tile([BA, 2], F16)
            nc.scalar.dma_start(out=hmask[:], in_=hmask_d[:])
            # touch Ln so its activation table loads during startup rather
            # than on the stage-2 critical path
            lnwarm = per.tile([BA, 1], F16)
            nc.scalar.activation(out=lnwarm[:], in_=hmask[:, 0:1], func=AF.Ln)
            w_t = per.tile([J, H, T], F16)
            nc.scalar.dma_start(out=w_t[:], in_=wgt_d[:])
            r_t = per.tile([J, H, T], F16)
            nc.scalar.dma_start(out=r_t[:], in_=rwd_d[:])

            # s2d[j, g, rho, t]: per-(ba,t) sums in stage-2 layout
            s2d = per.tile([J, H, 6, T], F16)

            # ---- stage 1: stream big chunks, compute per FCH --------------
            for cb in range(NCB):
                bsl = slice(cb * CB, (cb + 1) * CB)

                # host-precomputed onehot first: it gates three of the
                # four products, so it must land before the bulk loads
                oh = inp.tile([BA, CB], F16, tag="oh")
                lg = inp.tile([BA, CB], F16, tag="lg")
                qt = inp.tile([BA, CB], F16, tag="qt")
                tq = inp.tile([BA, CB], F16, tag="tq")
                if cb == 0:
                    # split the first loads so chunk 0's compute starts
                    # as soon as its halves land
                    nc.sync.dma_start(out=oh[:, 0:FCH], in_=ohf_d[:, 0:FCH])
                    nc.sync.dma_start(out=qt[:, 0:FCH], in_=qv_d[:, 0:FCH])
                    nc.sync.dma_start(out=lg[:, 0:FCH], in_=lg_d[:, 0:FCH])
                    nc.sync.dma_start(out=tq[:, 0:FCH], in_=tqv_d[:, 0:FCH])
                    nc.sync.dma_start(out=oh[:, FCH:CB], in_=ohf_d[:, FCH:CB])
                    nc.sync.dma_start(out=qt[:, FCH:CB], in_=qv_d[:, FCH:CB])
                    nc.sync.dma_start(out=lg[:, FCH:CB], in_=lg_d[:, FCH:CB])
                    nc.sync.dma_start(out=tq[:, FCH:CB], in_=tqv_d[:, FCH:CB])
                else:
                    nc.sync.dma_start(out=oh[:], in_=ohf_d[:, bsl])
                    nc.sync.dma_start(out=lg[:], in_=lg_d[:, bsl])
                    nc.sync.dma_start(out=qt[:], in_=qv_d[:, bsl])
                    nc.sync.dma_start(out=tq[:], in_=tqv_d[:, bsl])

                # evac tile covers a full big-chunk (2 compute chunks);
                # free dims = (wave 2, j_loc 16, t 256).  Rows: {32c, 32c+1};
                # wave1 products in wave-slot 0, wave2 in wave-slot 1.
                scc1 = evc.tile([98, 2, 16, T], F16, tag="scc1")

                for ci in range(2):
                    c = 2 * cb + ci
                    csl = slice(ci * FCH, (ci + 1) * FCH)

                    e = scr.tile([BA, FCH], F16, tag="e")
                    nc.scalar.activation(out=e[:], in_=lg[:, csl], func=AF.Exp)

                    gq = scr.tile([BA, FCH], F16, tag="gq")
                    nc.vector.tensor_mul(gq[:], oh[:, csl], qt[:, csl])
                    gtq = scr.tile([BA, FCH], F16, tag="gtq")
                    nc.vector.tensor_mul(gtq[:], oh[:, csl], tq[:, csl])
                    glg = scr.tile([BA, FCH], F16, tag="glg")
                    nc.vector.tensor_mul(glg[:], oh[:, csl], lg[:, csl])
                    peq = scr.tile([BA, FCH], F16, tag="peq")
                    nc.vector.tensor_mul(peq[:], e[:], qt[:, csl])
                    pel = scr.tile([BA, FCH], F16, tag="pel")
                    nc.vector.tensor_mul(pel[:], e[:], lg[:, csl])

                    # (product, psum row base, col offset, array col group):
                    # wave1 in psum cols 0-1023, wave2 in cols 1024-2047
                    waves = [
                        (e, 0, 0, 0), (peq, 32, 0, 32),
                        (gq, 64, 0, 64), (gtq, 96, 0, 96),
                        (pel, 0, 1024, 0), (glg, 32, 1024, 32),
                    ]
                    for hf in range(2):  # half-chunks of 1024 cols
                        p1 = ps1.tile([98, 2048], F32, tag="p1")
                        for s2 in range(2):  # SUBs of 512
                            ssl = slice((2 * hf + s2) * SUB, (2 * hf + s2 + 1) * SUB)
                            for prod, rb, co, cg in waves:
                                psl = slice(co + s2 * SUB, co + (s2 + 1) * SUB)
                                nc.tensor.matmul(
                                    out=p1[rb : rb + 2, psl],
                                    lhsT=hmask[:],
                                    rhs=prod[:, ssl],
                                    start=True,
                                    stop=True,
                                    tile_position=(0, cg),
                                )
                        # evacuate this half-chunk into quarter q of the
                        # big-chunk evac tile (one copy: both wave slots;
                        # GPSIMD cannot access PSUM, so scalar does these)
                        q = 2 * ci + hf
                        if c == NCH - 1:
                            # the last chunk's evacs run on DVE (idle by
                            # then) so the drain doesn't sit behind the
                            # scalar evac backlog
                            nc.vector.tensor_copy(
                                scc1[:, :, 4 * q : 4 * q + 4, :], p1[:]
                            )
                        else:
                            nc.scalar.activation(
                                out=scc1[:, :, 4 * q : 4 * q + 4, :],
                                in_=p1[:],
                                func=AF.Copy,
                            )

                    # bounce the 12 data rows to DRAM in stage-2 element
                    # order sct[j_loc, g, rho, t], then repack to s2d.
                    # Done per compute chunk so the final drain
                    # is short.  These wait on evac semaphores, so they go
                    # on the gpsimd queue — on sync they would block the
                    # next big-chunk's input prefetch (in-order queue).
                    jsl = slice(8 * ci, 8 * ci + 8)
                    sct = drb.tile([8, H, 6, T], F16, tag=f"sct{c}")
                    # the last big-chunk's g=1 bounces go on sync (idle by
                    # then, and no later input loads to block) to halve the
                    # serial drain on the gpsimd queue
                    g1eng = nc.sync if cb == NCB - 1 else nc.gpsimd
                    for g, beng in ((0, nc.gpsimd), (1, g1eng)):
                        beng.dma_start(
                            out=sct[:, g, 0:4, :].transpose([1, 0, 2]),
                            in_=scc1[g : 98 : 32, 0, jsl, :],
                        )
                        beng.dma_start(
                            out=sct[:, g, 4:6, :].transpose([1, 0, 2]),
                            in_=scc1[g : 33 + g : 32, 1, jsl, :],
                        )
                    nc.gpsimd.dma_start(
                        out=s2d[c * 8 : (c + 1) * 8, :, :, :], in_=sct[:]
                    )

            # re-warm the Ln activation table after the last Exp so stage 2
            # doesn't pay the table load on its critical path
            nc.scalar.activation(out=lnwarm[:], in_=hmask[:, 0:1], func=AF.Ln)

            # ---- stage 2: merged-h ops on [J, 2, T] slices ----------------
            def S(rho):
                return s2d[:, :, rho, :]

            # lambda returns per half first: independent of the z/rs chain
            d = per.tile([J, H, T - 1], F16)
            nc.vector.tensor_scalar_mul(
                d[:], S(R_TQTK)[:, :, 1:T], GAMMA * (1.0 - LAMBDA)
            )
            nc.vector.tensor_add(d[:], d[:], r_t[:, :, 0 : T - 1])
            gl = per.tile([J, 1], F16)
            nc.vector.memset(gl[:], GAMMA * LAMBDA)
            ret = per.tile([J, H, T - 1], F16)
            for h in range(H):
                nc.vector.tensor_tensor_scan(
                    out=ret[:, h, ::-1],
                    data0=gl[:].to_broadcast([J, T - 1]),
                    data1=d[:, h, ::-1],
                    initial=s2d[:, h, R_TQTK, T - 1 : T],
                    op0=OP.mult,
                    op1=OP.add,
                )

            z = per.tile([J, H, T], F16)
            nc.scalar.activation(out=z[:], in_=S(R_SUME), func=AF.Ln)
            se32 = per.tile([J, H, T], F32)
            nc.vector.tensor_copy(se32[:], S(R_SUME))
            rs = per.tile([J, H, T], F32)
            nc.vector.reciprocal_approx_fast(rs[:], se32[:])

            logp = per.tile([J, H, T], F16)
            nc.vector.tensor_tensor(out=logp[:], in0=S(R_LTK), in1=z[:], op=OP.subtract)
            bl = per.tile([J, H, T], F16)
            nc.vector.tensor_mul(bl[:], S(R_DOTEQ), rs[:])
            adv = per.tile([J, H, T], F16)
            nc.vector.tensor_tensor(out=adv[:], in0=S(R_QTK), in1=bl[:], op=OP.subtract)
            ent = per.tile([J, H, T], F16)
            nc.vector.tensor_mul(ent[:], S(R_DOTEL), rs[:])
            nc.vector.tensor_tensor(out=ent[:], in0=z[:], in1=ent[:], op=OP.subtract)

            pol = per.tile([J, H, T], F16)
            nc.vector.tensor_mul(pol[:], logp[:], adv[:])
            nc.vector.tensor_mul(pol[:], pol[:], w_t[:])
            entw = per.tile([J, H, T], F16)
            nc.vector.tensor_mul(entw[:], ent[:], w_t[:])

            qd = per.tile([J, H, T - 1], F16)
            nc.vector.tensor_tensor(
                out=qd[:], in0=ret[:], in1=S(R_QTK)[:, :, 0 : T - 1], op=OP.subtract
            )
            nc.vector.tensor_mul(qd[:], qd[:], qd[:])
            nc.vector.tensor_mul(qd[:], qd[:], w_t[:, :, 0 : T - 1])

            partials = per.tile([J, 3], F32)
            dump = per.tile([J, H, T], F16)
            nc.scalar.activation(
                out=dump[:], in_=pol[:], func=AF.Copy, accum_out=partials[:, 0:1]
            )
            nc.scalar.activation(
                out=dump[:, :, 0 : T - 1], in_=qd[:],
                func=AF.Copy, accum_out=partials[:, 1:2],
            )
            nc.scalar.activation(
                out=dump[:], in_=entw[:], func=AF.Copy, accum_out=partials[:, 2:3]
            )
            nc.sync.dma_start(out=out_d[:], in_=partials[:])

    return nc


def make_in_maps(logit, action, q_value, target_q_value, reward, weight):
    """Shard + marshal full inputs into per-core input dicts."""
    logit = np.asarray(logit, np.float32)
    q_value = np.asarray(q_value, np.float32)
    target_q_value = np.asarray(target_q_value, np.float32)
    action = np.asarray(action)
    reward = np.asarray(reward, np.float32)
    weight = np.asarray(weight, np.float32)

    hmask = np.zeros((BA, 2), np.float16)
    hmask[:J, 0] = 1.0
    hmask[J:, 1] = 1.0

    in_maps = []
    for r in range(M):
        bs, be = r * BL, (r + 1) * BL

        def big(x):
            # [T, BL, A, N] = [t, (h,j), n] -> [h, n, j, t] -> [128, F]
            y = x[:, bs:be].reshape(T, 2, J, N).transpose(1, 3, 2, 0)
            return np.ascontiguousarray(y).reshape(BA, F).astype(np.float16)

        # onehot over n: ohf[64h+n, j*T+t] = (action[t, h, j] == n)
        act_c = action[:, bs:be].reshape(T, 2, J).transpose(1, 2, 0)  # [h,j,t]
        ohf = (
            act_c[:, None, :, :] == np.arange(N)[None, :, None, None]
        ).reshape(BA, F).astype(np.float16)

        def small(x):
            # [T, 128] -> [j, h, t]
            y = x.reshape(T, 2, J).transpose(2, 1, 0)
            return np.ascontiguousarray(y).reshape(J, H * T).astype(np.float16)

        in_maps.append(
            {
                "logit": big(logit),
                "qv": big(q_value),
                "tqv": big(target_q_value),
                "ohf": ohf,
                "hmask": hmask,
                "wgt": small(weight[:, bs:be].reshape(T, BA)),
                "rwd": small(np.repeat(reward[:, bs:be], A, axis=1)),
            }
        )
    return in_maps


def combine_partials(partials_per_core):
    """[M][64, 3] partial sums -> the three scalar losses."""
    s = np.stack(partials_per_core).astype(np.float64).sum(axis=(0, 1))
    pol, qd, ent = s[0], s[1], s[2]
    policy_loss = np.float32(-pol / (T * B * A))
    q_value_loss = np.float32(qd / ((T - 1) * B * A))
    entropy_loss = np.float32(ent / (T * B * A))
    return policy_loss, q_value_loss, entropy_loss


_program_cache = {}


def _get_program() -> bass.Bass:
    if "nc" not in _program_cache:
        nc = build_program()
        nc.finalize()
        _program_cache["nc"] = nc
    return _program_cache["nc"]


def kernel(logit, action, q_value, target_q_value, reward, weight):
    nc = _get_program()
    in_maps = make_in_maps(logit, action, q_value, target_q_value, reward, weight)
    res = run_bass_kernel_spmd(nc, in_maps, list(range(M))).results
    return combine_partials(
        [np.asarray(res[i]["out"]).reshape(J, 3) for i in range(M)]
    )
